# revision 40
# baseline (speedup 1.0000x reference)
"""Causal self-attention (B=1, T=4096, C=1024, H=16) on 8 trn2 NeuronCores.

Sharding: tensor-parallel over heads - 2 heads per core. Each core computes
q/k/v for its 2 heads from the full sequence, runs causal attention fully
on-chip, and produces a partial output projection (its heads' contribution
y_h @ W_proj[head_rows]); the host sums the 8 partials; b_proj is added
during the host-side unshard.

Built around two measured PE facts:
  - the HAM clock gate runs the PE at 1.2 GHz until ~3.4us of sustained
    activity (then 2.4 GHz), so the whole kernel is ONE software-
    pipelined matmul stream with no intentional PE idle;
  - transitions between K=64 and K=128 weight loads cost ~105-160ns
    each (FWL reconfig), so kT is stored as TWO zero-padded per-head
    tiles (kT_h = [k_h; 0]) making every S matmul a full K=128 matmul.
    With that, back-to-back 512-col matmuls sustain the warm 216ns.

Per-core layouts:
  qT          [128, T] bf16 (both heads stacked; 1/sqrt(hd) folded in)
  kT0/kT1     [128, T] bf16, other head's 64 rows zeroed
  v           [T, .] bf16, per-l-tile slots [v_h0|1|pad|v_h1|1|pad]; the
              constant-1 columns make the P@V matmul also emit the
              softmax denominators (row 64 of each head's [65,512] psum)
  S^T         [l, q] per (l-tile-pair, head): two K=128 matmuls into one
              [128,1024] psum pair-tile, exp'd by ONE ACT op (ACT has
              ~250-350ns fixed per-op cost, so pair-size exps beat
              per-tile ones); no max-subtraction (|logits| <~ 9). For
              diagonal pairs the second tile's S is NOT col-trimmed so
              the psum gap holds real (unread) values and one exp still
              covers the pair; the upper-triangle of diagonal 128-col
              blocks is zeroed by Pool affine_select after the exp.
  oT          [d=128, q] accumulated in psum; denominators inverted on
              ACT as exp(-ln den) into bf16 (DVE reciprocal measures
              3.3us; no DVE divide exists), broadcast across partitions
              by a K=1 ones-matmul on the PE (~0.2us) + DVE copy to
              SBUF (~2.5us shorter chain than a DRAM-roundtrip DMA),
              and applied by one DVE mult per head.

Pipelining: the job unit is one (pair, head): S x2 -> exp -> PV x2, with
S+exp emitted one pair ahead of the PVs (st pool = 2x2 psum banks).
Pairs are head-blocked in groups of 4 l-tiles so each super's first h1
PV lands 2 pairs in, and h0's denominator chain starts 2 pairs before
the super ends. Non-attention matmuls are FILLERS between pairs of
super j:
  iter 0:   v(j) x8 [qkv bank] - all before the first PV, covering the
            previous super's den/norm chain (ot pool bufs=2 WAR)
  iter 1:   k(j) x8 [pj bank]
  iter 2:   transposes(j) x4 [qkv bank, after the v-bias drain]
  iter 3+:  q(j+1) x8 spread [qkv bank, after the tp-copy drain]
  iter 4+:  proj(j-1) x8 spread [pj bank; the 1-bank rotation's DVE-cast
            WAR is covered by spacing]
so per-iteration PE work stays above ACT's ~1.05us pair-exp (the pacing
op in late supers). The last super's norm is split by 128-col blocks,
interleaved with its projection matmuls (casts alternate DVE/ACT, psum
alternates pj/st banks) to shorten the tail. Output partials are stored
bf16 (the host accumulates f32).

PSUM budget (8 banks): st 2x2 + ot 2 + qkv 1 + pj 1.
"""

import numpy as np
from contextlib import ExitStack

import concourse.bass as bass
import concourse.mybir as mybir
import concourse.tile as tile
from concourse.bass import AP
from concourse.masks import make_identity

T = 4096
C = 1024
H = 16
HD = 64
NCORES = 8
SUP = 512           # q super-block width
NSUP = T // SUP
LTN = T // 128      # number of 128-row l-tiles
VSLOT = 130         # v slot: [v_h0(0:64)|1(64)|v_h1(65:129)|1(129)]

F32 = mybir.dt.float32
BF16 = mybir.dt.bfloat16
AF = mybir.ActivationFunctionType
ALU = mybir.AluOpType


def _split_multi_waits(nc, max_waits=1):
    """The walrus build here rejects >1 semaphore wait on one CTRL
    instruction; push excess waits onto preceding same-engine NoOps."""
    n_new = 0
    for f in nc.m.functions:
        for bb in f.blocks:
            out = []
            changed = False
            for ins in bb.instructions:
                si = ins.sync_info
                waits = list(si.on_wait) if si is not None else []
                if len(waits) > max_waits:
                    changed = True
                    excess, keep = waits[:-max_waits], waits[-max_waits:]
                    for ci in range(0, len(excess), max_waits):
                        n_new += 1
                        out.append(mybir.InstNoOp(
                            name=f"{ins.name}-ws{n_new}",
                            engine=ins.engine, ins=[], outs=[],
                            sync_info=mybir.SyncInfo(
                                on_wait=excess[ci:ci + max_waits], on_update=[]),
                        ))
                    ins.sync_info = mybir.SyncInfo(
                        on_wait=keep, on_update=list(si.on_update))
                out.append(ins)
            if changed:
                bb.instructions = out
    return n_new


def build_nc(split_waits=True):
    nc = bass.Bass("TRN2")
    xT = nc.dram_tensor("xT", [C, T], BF16, kind="ExternalInput")
    wq = nc.dram_tensor("wq", [C, 128], BF16, kind="ExternalInput")
    wk = nc.dram_tensor("wk", [C, 128], BF16, kind="ExternalInput")
    wv = nc.dram_tensor("wv", [C, 128], BF16, kind="ExternalInput")
    bq = nc.dram_tensor("bq", [128, 1], F32, kind="ExternalInput")
    bk = nc.dram_tensor("bk", [128, 1], F32, kind="ExternalInput")
    bv = nc.dram_tensor("bv", [128, 1], F32, kind="ExternalInput")
    wp = nc.dram_tensor("wp", [128, C], BF16, kind="ExternalInput")
    out_d = nc.dram_tensor("out", [T, C], BF16, kind="ExternalOutput")

    with tile.TileContext(nc) as tc:
        with ExitStack() as ctx:
            P = lambda **kw: ctx.enter_context(tc.tile_pool(**kw))
            const_p = P(name="const", bufs=1)
            qk_p = P(name="qk", bufs=1)
            v_p = P(name="v", bufs=1)
            x_p = P(name="x", bufs=5)
            vt_p = P(name="vt", bufs=2)
            pt_p = P(name="pt", bufs=6)
            ot_sb_p = P(name="ot_sb", bufs=4)
            ep_p = P(name="ep", bufs=4)
            rl_p = P(name="rl", bufs=2)

            # ---- constants ----
            warm_sb = const_p.tile([128, 64], BF16)
            nc.gpsimd.memset(warm_sb[:], 0.0)
            wq_sb = const_p.tile([128, 8, 128], BF16)
            wk_sb = const_p.tile([128, 8, 128], BF16)
            wv_sb = const_p.tile([128, 8, 128], BF16)
            bq_sb = const_p.tile([128, 1], F32)
            bk_sb = const_p.tile([128, 1], F32)
            bv_sb = const_p.tile([128, 1], F32)
            x_tiles = {}

            def fetch_x(s):
                x_sb = x_p.tile([128, 8, SUP], BF16)
                nc.sync.dma_start(
                    x_sb[:],
                    xT[:, s * SUP:(s + 1) * SUP].rearrange(
                        "(ck p) t -> p ck t", p=128))
                x_tiles[s] = x_sb

            # v-phase runs first, so wv + x(0) must land first; weights go
            # on the ACT queue so their setups overlap the x fetches on the
            # sync queue. x(0) fetched in halves so QKV can start early.
            nc.scalar.dma_start(
                wv_sb[:], wv[:].rearrange("(ck p) m -> p ck m", p=128))
            nc.gpsimd.dma_start(bv_sb[:], bv[:])
            x0_sb = x_p.tile([128, 8, SUP], BF16)
            for ck in range(8):
                nc.sync.dma_start(
                    x0_sb[:, ck:ck + 1, :],
                    xT[ck * 128:(ck + 1) * 128, 0:SUP].rearrange(
                        "(ck p) t -> p ck t", p=128))
            x_tiles[0] = x0_sb
            nc.scalar.dma_start(
                wq_sb[:], wq[:].rearrange("(ck p) m -> p ck m", p=128))
            nc.gpsimd.dma_start(bq_sb[:], bq[:])
            nc.scalar.dma_start(
                wk_sb[:], wk[:].rearrange("(ck p) m -> p ck m", p=128))
            nc.gpsimd.dma_start(bk_sb[:], bk[:])
            fetch_x(1)
            wp_sb = const_p.tile([128, C], BF16)
            nc.gpsimd.dma_start(wp_sb[:], wp[:])
            ident = const_p.tile([128, 128], BF16)
            make_identity(nc, ident[:])
            tblw = const_p.tile([1, 64], F32)
            nc.scalar.activation(tblw[:], warm_sb[0:1, 0:64], AF.Exp)
            ones_bf = const_p.tile([1, 64], BF16)
            nc.gpsimd.memset(ones_bf[:], 1.0)
            v_sb = v_p.tile([128, LTN * VSLOT], BF16)
            nc.gpsimd.memset(v_sb[:], 1.0)  # ones cols survive the transposes

            qT = qk_p.tile([128, T], BF16)
            # per-head kT, zero-padded in the other head's 64 rows, so the
            # S matmul is a full K=128 matmul: transitions between K=64
            # and K=128 weight loads cost ~105-160ns each on the PE (FWL
            # reconfig); with K=128 everywhere every back-to-back matmul
            # runs at the warm ~216ns/512col rate.
            kT0 = qk_p.tile([128, T], BF16)
            kT1 = qk_p.tile([128, T], BF16)
            nc.gpsimd.memset(kT0[64:128, :], 0.0)
            nc.gpsimd.memset(kT1[0:64, :], 0.0)

            st_ps = P(name="st_ps", bufs=2, space="PSUM")
            ot_ps_p = P(name="ot_ps", bufs=2, space="PSUM")
            qkv_ps = P(name="qkv_ps", bufs=1, space="PSUM")
            pj_ps = P(name="pj_ps", bufs=1, space="PSUM")

            qkv_state = {}   # per-super QKV phase state

            def v_thunks(s):
                st = qkv_state.setdefault(s, {})

                def mk(ck):
                    def f():
                        if ck == 0:
                            st["v"] = qkv_ps.tile([128, SUP], F32, tag="qkv",
                                                  name=f"psv{s}")
                        nc.tensor.matmul(
                            st["v"][:], lhsT=wv_sb[:, ck, :],
                            rhs=x_tiles[s][:, ck, :],
                            start=(ck == 0), stop=(ck == 7))
                        if ck == 7:
                            st["vt"] = vt_p.tile([128, SUP], BF16, tag="vt",
                                                 name=f"vt{s}")
                            nc.vector.tensor_scalar_add(
                                out=st["vt"][:], in0=st["v"][:],
                                scalar1=bv_sb[:])
                    return f
                return [mk(c) for c in range(8)]

            def q_thunks(s, dense0=False):
                st = qkv_state.setdefault(s, {})

                def mk(ck):
                    def f():
                        if ck == 0:
                            pool, tg = ((st_ps, "st") if dense0
                                        else (qkv_ps, "qkv"))
                            st["q"] = pool.tile([128, SUP], F32, tag=tg,
                                                name=f"psq{s}")
                        nc.tensor.matmul(
                            st["q"][:], lhsT=wq_sb[:, ck, :],
                            rhs=x_tiles[s][:, ck, :],
                            start=(ck == 0), stop=(ck == 7))
                        if ck == 7:
                            # (q + bias) * 1/sqrt(hd) folded here
                            nc.vector.tensor_scalar(
                                out=qT[:, s * SUP:(s + 1) * SUP],
                                in0=st["q"][:], scalar1=bq_sb[:],
                                scalar2=1.0 / np.sqrt(HD),
                                op0=ALU.add, op1=ALU.mult)
                    return f
                return [mk(c) for c in range(8)]

            def k_thunks(s):
                st = qkv_state.setdefault(s, {})

                def mk(ck):
                    def f():
                        if ck == 0:
                            st["k"] = pj_ps.tile([128, SUP], F32, tag="pj",
                                                 name=f"psk{s}")
                        nc.tensor.matmul(
                            st["k"][:], lhsT=wk_sb[:, ck, :],
                            rhs=x_tiles[s][:, ck, :],
                            start=(ck == 0), stop=(ck == 7))
                        if ck == 7:
                            nc.vector.tensor_scalar_add(
                                out=kT0[0:64, s * SUP:(s + 1) * SUP],
                                in0=st["k"][0:64, :], scalar1=bk_sb[0:64])
                            nc.vector.tensor_scalar_add(
                                out=kT1[64:128, s * SUP:(s + 1) * SUP],
                                in0=st["k"][64:128, :],
                                scalar1=bk_sb[64:128])
                            x_tiles.pop(s)
                    return f
                return [mk(c) for c in range(8)]

            def tp_thunks(s, dense0=False):
                st = qkv_state.setdefault(s, {})

                def mk(lt_loc):
                    def f():
                        if lt_loc == 0:
                            pool, tg = ((st_ps, "st") if dense0
                                        else (qkv_ps, "qkv"))
                            st["tp"] = pool.tile([128, SUP], BF16, tag=tg,
                                                 name=f"tp{s}")
                        lt = s * 4 + lt_loc
                        blk = slice(lt_loc * 128, (lt_loc + 1) * 128)
                        nc.tensor.transpose(
                            st["tp"][:, blk], st["vt"][:, blk], ident[:])
                        nc.vector.tensor_copy(
                            v_sb[:, lt * VSLOT: lt * VSLOT + 64],
                            st["tp"][:, lt_loc * 128: lt_loc * 128 + 64])
                        nc.vector.tensor_copy(
                            v_sb[:, lt * VSLOT + 65: lt * VSLOT + 129],
                            st["tp"][:, lt_loc * 128 + 64:
                                      lt_loc * 128 + 128])
                        if lt_loc == 3:
                            qkv_state.pop(s, None)
                    return f
                return [mk(b) for b in range(4)]

            def proj_thunks(jj, ot_sb, alternate=False):
                """Projection for super jj: 8 single-MM thunks, each with
                its cast + store. One psum bank; spacing covers the WAR.
                alternate=True (epilogue): rotate a 2nd (st) bank and split
                casts DVE/ACT so the tail isn't serialized on one drain."""
                def mk(idx, tb, half):
                    def f():
                        if alternate and idx % 2:
                            pj = st_ps.tile([128, 512], F32, tag="st",
                                            name=f"pj{jj}_{tb}_{half}")
                        else:
                            pj = pj_ps.tile([128, 512], F32, tag="pj",
                                            name=f"pj{jj}_{tb}_{half}")
                        nc.tensor.matmul(
                            pj[:],
                            lhsT=ot_sb[:, tb * 128:(tb + 1) * 128],
                            rhs=wp_sb[:, half * 512:(half + 1) * 512],
                            start=True, stop=True)
                        res = ep_p.tile([128, 512], BF16, tag="res",
                                        name=f"res{jj}_{tb}_{half}")
                        if alternate and idx % 2:
                            nc.scalar.activation(res[:], pj[:], AF.Copy)
                        else:
                            nc.vector.tensor_copy(res[:], pj[:])
                        nc.sync.dma_start(
                            out_d[jj * SUP + tb * 128:
                                  jj * SUP + (tb + 1) * 128,
                                  half * 512:(half + 1) * 512],
                            res[:])
                    return f
                return [mk(i, i // 2, i % 2) for i in range(8)]

            # ---- HAM warm-up: dependency-free tiny matmuls fill the
            # initial x-chunk DMA gaps so the PE clock gate reaches K=8/8
            # (2.4 GHz) before the main QKV stream instead of ~3.4us in;
            # interleaved between the v thunks so they plug per-chunk
            # DMA waits rather than delaying ready work ----
            warm_ps = pj_ps.tile([64, 64], F32, tag="pj", name="warm_ps")

            def warm(nmm):
                for wi in range(nmm):
                    nc.tensor.matmul(warm_ps[:], lhsT=warm_sb[:, 0:64],
                                     rhs=warm_sb[:, 0:64],
                                     start=True, stop=True)

            # ---- prologue: QKV(0) dense (q/tp borrow st banks so the
            # qkv bank has no WAR stalls), then prefetch ----
            warm(10)
            vth = v_thunks(0)
            for i, th in enumerate(vth):
                th()
                if i < 5:
                    warm(6)
            for th in (q_thunks(0, dense0=True) + k_thunks(0)
                       + tp_thunks(0, dense0=True)):
                th()
            fetch_x(2)
            fetch_x(3)

            ot_sb_tiles = {}     # j -> normalized [128, SUP] bf16
            pending_norm = {}    # j -> thunk emitting the 2 norm mults
            pending_bc = {}      # j -> thunk emitting h1's bc matmul+copy

            for j in range(NSUP):
                nlt = 4 * j + 4
                # head-blocked pair order: 2 l-tile pairs per head block.
                # Pair granularity on the exp: ACT has ~250-350ns of fixed
                # per-op overhead, so one [128,1024] exp beats two
                # [128,512] ones by ~300ns.
                pairs = []
                # head-blocks of 8 l-tiles (plus a remainder block of 4):
                # bigger blocks push each super's first h1 PV later (more
                # cover for the previous super's norm chain) and start h0's
                # den earlier.
                blks = [4] * (nlt // 4)
                t0b = 0
                for hb_ in blks:
                    for h in range(2):
                        for q2 in range(hb_ // 2):
                            pairs.append((t0b + 2 * q2, h))
                    t0b += hb_
                n = len(pairs)
                hb = blks[-1]
                last = j == NSUP - 1

                ot_ps = [ot_ps_p.tile([128, SUP], F32, tag="ot",
                                      name=f"ot{j}_{hh}") for hh in range(2)]
                rcs = [rl_p.tile([1, SUP], BF16, tag=f"rc{hh}",
                                 name=f"rc{j}_{hh}") for hh in range(2)]
                lns = [rl_p.tile([1, SUP], F32, tag=f"ln{hh}",
                                 name=f"ln{j}_{hh}") for hh in range(2)]
                bc_sb = rl_p.tile([128, SUP], BF16, tag="bc_sb",
                                  name=f"bc_sb{j}")
                ot_sb = ot_sb_p.tile([128, SUP], BF16, tag="ot_sb",
                                     name=f"ot_sb{j}")
                ot_sb_tiles[j] = ot_sb

                # ---- filler schedule for this super ----
                per = [[] for _ in range(n)]
                fillers = []

                def sched(thunks, lo, hi=None, dense=None):
                    """Spread thunks over iters lo..hi (dense = per-iter)."""
                    hi = n if hi is None else hi
                    base = len(fillers)
                    fillers.extend(thunks)
                    m = len(thunks)
                    if dense:
                        for i in range(m):
                            per[min(lo + i // dense, n - 1)].append(base + i)
                    else:
                        for i in range(m):
                            per[min(lo + i * (hi - lo) // m, n - 1)].append(
                                base + i)

                if j >= 1:
                    sched(v_thunks(j), 0, dense=8)     # iter 0, all slot-a
                    sched(k_thunks(j), 1, dense=8)     # iter 1 (pj bank)
                    sched(tp_thunks(j), 2, dense=4)    # iter 2
                if j + 1 < NSUP:
                    sched(q_thunks(j + 1), 3 if j >= 1 else 0)
                if j >= 1:
                    sched(proj_thunks(j - 1, ot_sb_tiles.pop(j - 1)), 4)

                def emit_S(p, idx):
                    t0, h = pairs[p]
                    t = t0 + idx
                    # second tile of a diagonal pair is NOT trimmed: the
                    # extra S cols land in the psum gap so one exp covers
                    # the whole pair (ACT per-op overhead ~0.7us dwarfs the
                    # ~0.1-0.2us of extra PE cols); the gap region is never
                    # read by PV.
                    n0 = max(0, 128 * (t - 4 * j)) if idx == 0 else 0
                    if idx == 0:
                        pair_state[p] = [
                            st_ps.tile([128, 2 * SUP], F32, tag="st",
                                       name=f"st{j}_{t0}_{h}"), None]
                    stt = pair_state[p][0]
                    kTh = kT0 if h == 0 else kT1
                    nc.tensor.matmul(
                        stt[:, idx * SUP + n0:(idx + 1) * SUP],
                        lhsT=kTh[:, t * 128:(t + 1) * 128],
                        rhs=qT[:, j * SUP + n0:(j + 1) * SUP],
                        start=True, stop=True)

                def emit_exp(p):
                    t0, h = pairs[p]
                    stt = pair_state[p][0]
                    pt = pt_p.tile([128, 2 * SUP], BF16, tag="pt",
                                   name=f"pt{j}_{t0}_{h}")
                    pair_state[p][1] = pt
                    e0 = max(0, 128 * (t0 - 4 * j))
                    nc.scalar.activation(
                        pt[:, e0:2 * SUP], stt[:, e0:2 * SUP], AF.Exp)
                    for idx in range(2):
                        t = t0 + idx
                        if t >= 4 * j:
                            # zero strictly-upper triangle of the diagonal
                            # 128-col block: keep col >= part
                            n0 = 128 * (t - 4 * j)
                            nc.gpsimd.affine_select(
                                out=pt[:, idx * SUP + n0:
                                       idx * SUP + n0 + 128],
                                in_=pt[:, idx * SUP + n0:
                                       idx * SUP + n0 + 128],
                                compare_op=ALU.is_ge, fill=0.0, base=0,
                                channel_multiplier=-1, pattern=[[1, 128]])

                def emit_PV(p, idx):
                    t0, h = pairs[p]
                    t = t0 + idx
                    n0 = max(0, 128 * (t - 4 * j))
                    pt = pair_state[p][1]
                    nc.tensor.matmul(
                        ot_ps[h][0:65, n0:SUP],
                        lhsT=v_sb[:, t * VSLOT + h * 65:
                                  t * VSLOT + (h + 1) * 65],
                        rhs=pt[:, idx * SUP + n0:(idx + 1) * SUP],
                        start=(t == 0), stop=(t == 4 * j + 3))

                def den_chain(h):
                    # 1/den as exp(-ln den) on ACT (~0.8us/op; Ln/Exp share
                    # one table; DVE reciprocal measures 3.3us and a DVE
                    # divide does not exist on this HW). rc is written bf16
                    # for the PE ones-matmul broadcast.
                    nc.scalar.activation(lns[h][:], ot_ps[h][64:65, :],
                                         AF.Ln)
                    nc.scalar.activation(rcs[h][:], lns[h][:], AF.Exp,
                                         scale=-1.0)

                def bc_mm(h, bct, rcs_, bc_sb_):
                    # broadcast 1/den across partitions with a K=1 bf16
                    # ones-matmul (~0.2us on PE) + DVE copy to SBUF -
                    # ~2.5us shorter chain than a DRAM-roundtrip DMA.
                    nc.tensor.matmul(
                        bct[h * 64:(h + 1) * 64, :], lhsT=ones_bf[:],
                        rhs=rcs_[h][:], start=True, stop=True,
                        tile_position=(0, h * 64))
                    nc.vector.tensor_copy(
                        bc_sb_[h * 64:(h + 1) * 64, :],
                        bct[h * 64:(h + 1) * 64, :])

                def make_norm(jj, ot_ps_, bc_, ot_sb_):
                    def f():
                        for h in range(2):
                            nc.vector.tensor_tensor(
                                out=ot_sb_[h * 64:(h + 1) * 64, :],
                                in0=ot_ps_[h][0:64, :],
                                in1=bc_[h * 64:(h + 1) * 64, :],
                                op=ALU.mult)
                    return f

                pair_state = {}
                emit_S(0, 0)
                emit_S(0, 1)
                emit_exp(0)
                for p in range(n):
                    if p == 0 and j + 4 < NSUP:
                        fetch_x(j + 4)
                    if p == 1 and j >= 1:
                        pending_norm.pop(j - 1)()
                    fa = per[p]
                    # iter 0: all fillers before the PVs - they are the PE
                    # cover for the previous super's den/norm chain, which
                    # the first PVs (ot-bank WAR) wait on
                    half = len(fa) if p == 0 else (len(fa) + 1) // 2
                    # full S pair + exp first so ACT gets a ~full-iter
                    # head start on exp(p+1) before PV(p+1,*) need it
                    if p + 1 < n:
                        emit_S(p + 1, 0)
                        emit_S(p + 1, 1)
                        emit_exp(p + 1)
                    for fi in fa[:half]:
                        fillers[fi]()
                    if p == 0 and j >= 1:
                        pending_bc.pop(j - 1)()
                    emit_PV(p, 0)
                    for fi in fa[half:]:
                        fillers[fi]()
                    emit_PV(p, 1)
                    pair_state.pop(p)
                    if p == n - 1 - hb // 2:
                        den_chain(0)
                    if p == n - 1:
                        den_chain(1)

                        def mk_bc(f=bc_mm, jj=j, r=rcs, b=bc_sb):
                            def g():
                                bct = pj_ps.tile([128, SUP], F32, tag="pj",
                                                 name=f"bct{jj}")
                                f(0, bct, r, b)
                                f(1, bct, r, b)
                            return g
                        pending_bc[j] = mk_bc()
                        pending_norm[j] = make_norm(j, ot_ps, bc_sb, ot_sb)

            # ---- epilogue: last super's norm + projection, interleaved
            # by 128-col blocks so each projection matmul starts as soon as
            # its slice of the normalize is done ----
            jl = NSUP - 1
            del pending_norm[jl]
            pending_bc.pop(jl)()
            pths = proj_thunks(jl, ot_sb_tiles.pop(jl), alternate=True)
            for tb in range(4):
                cs = slice(tb * 128, (tb + 1) * 128)
                for h in range(2):
                    nc.vector.tensor_tensor(
                        out=ot_sb[h * 64:(h + 1) * 64, cs],
                        in0=ot_ps[h][0:64, cs],
                        in1=bc_sb[h * 64:(h + 1) * 64, cs],
                        op=ALU.mult)
                pths[2 * tb]()
                pths[2 * tb + 1]()

    if split_waits:
        _split_multi_waits(nc, 1)
    return nc


_NC_CACHE = {}


def _get_nc():
    if "nc" not in _NC_CACHE:
        _NC_CACHE["nc"] = build_nc()
    return _NC_CACHE["nc"]


def make_in_maps(x, W_attn, b_attn, W_proj, b_proj):
    import ml_dtypes
    bf = ml_dtypes.bfloat16
    x = np.ascontiguousarray(np.asarray(x, dtype=np.float32)).reshape(T, C)
    W_attn = np.asarray(W_attn, dtype=np.float32)
    b_attn = np.asarray(b_attn, dtype=np.float32)
    W_proj = np.asarray(W_proj, dtype=np.float32)
    xT = np.ascontiguousarray(x.T).astype(bf)
    in_maps = []
    for c in range(NCORES):
        sl = slice(128 * c, 128 * (c + 1))
        m = {
            "xT": xT,
            "wq": np.ascontiguousarray(W_attn[:, sl]).astype(bf),
            "wk": np.ascontiguousarray(W_attn[:, C:][:, sl]).astype(bf),
            "wv": np.ascontiguousarray(W_attn[:, 2 * C:][:, sl]).astype(bf),
            "bq": np.ascontiguousarray(b_attn[sl]).reshape(128, 1),
            "bk": np.ascontiguousarray(b_attn[C:][sl]).reshape(128, 1),
            "bv": np.ascontiguousarray(b_attn[2 * C:][sl]).reshape(128, 1),
            "wp": np.ascontiguousarray(W_proj[sl, :]).astype(bf),
        }
        in_maps.append(m)
    return in_maps


def kernel(x, W_attn, b_attn, W_proj, b_proj):
    from concourse.bass_utils import run_bass_kernel_spmd
    nc = _get_nc()
    in_maps = make_in_maps(x, W_attn, b_attn, W_proj, b_proj)
    res = run_bass_kernel_spmd(nc, in_maps, core_ids=list(range(NCORES)))
    acc = np.zeros((T, C), dtype=np.float32)
    for c in range(NCORES):
        acc += np.asarray(res.results[c]["out"], dtype=np.float32)
    acc += np.asarray(b_proj, dtype=np.float32)  # bias folded into unshard
    return acc.reshape(1, T, C)


# revision 41
# speedup vs baseline: 1.2208x; 1.2208x over previous
"""Causal self-attention (B=1, T=4096, C=1024, H=16) on 8 trn2 NeuronCores.

Sharding: tensor-parallel over heads - 2 heads per core. Each core computes
q/k/v for its 2 heads from the full sequence, runs causal attention fully
on-chip, and produces a partial output projection (its heads' contribution
y_h @ W_proj[head_rows]); the host sums the 8 partials; b_proj is added
during the host-side unshard.

Built around two measured PE facts:
  - the HAM clock gate runs the PE at 1.2 GHz until ~3.4us of sustained
    activity (then 2.4 GHz), so the whole kernel is ONE software-
    pipelined matmul stream with no intentional PE idle;
  - transitions between K=64 and K=128 weight loads cost ~105-160ns
    each (FWL reconfig), so kT is stored as TWO zero-padded per-head
    tiles (kT_h = [k_h; 0]) making every S matmul a full K=128 matmul.
    With that, back-to-back 512-col matmuls sustain the warm 216ns.

Per-core layouts:
  qT          [128, T] bf16 (both heads stacked; 1/sqrt(hd) folded in)
  kT0/kT1     [128, T] bf16, other head's 64 rows zeroed
  v           [T, .] bf16, per-l-tile slots [v_h0|1|pad|v_h1|1|pad]; the
              constant-1 columns make the P@V matmul also emit the
              softmax denominators (row 64 of each head's [65,512] psum)
  S^T         [l, q] per (l-tile-pair, head): two K=128 matmuls into one
              [128,1024] psum pair-tile, exp'd by ONE ACT op (ACT has
              ~250-350ns fixed per-op cost, so pair-size exps beat
              per-tile ones); no max-subtraction (|logits| <~ 9). For
              diagonal pairs the second tile's S is NOT col-trimmed so
              the psum gap holds real (unread) values and one exp still
              covers the pair; the upper-triangle of diagonal 128-col
              blocks is zeroed by Pool affine_select after the exp.
  oT          [d=128, q] accumulated in psum; denominators inverted on
              ACT as exp(-ln den) into bf16 (DVE reciprocal measures
              3.3us; no DVE divide exists), broadcast across partitions
              by a K=1 ones-matmul on the PE (~0.2us) + DVE copy to
              SBUF (~2.5us shorter chain than a DRAM-roundtrip DMA),
              and applied by one DVE mult per head.

Pipelining: the job unit is one (pair, head): S x2 -> exp -> PV x2, with
S+exp emitted one pair ahead of the PVs (st pool = 2x2 psum banks).
Pairs are head-blocked in groups of 4 l-tiles so each super's first h1
PV lands 2 pairs in, and h0's denominator chain starts 2 pairs before
the super ends. Non-attention matmuls are FILLERS between pairs of
super j:
  iter 0:   v(j) x8 [qkv bank] - all before the first PV, covering the
            previous super's den/norm chain (ot pool bufs=2 WAR)
  iter 1:   k(j) x8 [pj bank]
  iter 2:   transposes(j) x4 [qkv bank, after the v-bias drain]
  iter 3+:  q(j+1) x8 spread [qkv bank, after the tp-copy drain]
  iter 4+:  proj(j-1) x8 spread [pj bank; the 1-bank rotation's DVE-cast
            WAR is covered by spacing]
so per-iteration PE work stays above ACT's ~1.05us pair-exp (the pacing
op in late supers). The last super's norm is split by 128-col blocks,
interleaved with its projection matmuls (casts alternate DVE/ACT, psum
alternates pj/st banks) to shorten the tail. Output partials are stored
bf16 (the host accumulates f32).

PSUM budget (8 banks): st 2x2 + ot 2 + qkv 1 + pj 1.
"""

import numpy as np
from contextlib import ExitStack

import concourse.bass as bass
import concourse.mybir as mybir
import concourse.tile as tile
from concourse.bass import AP
from concourse.masks import make_identity

T = 4096
C = 1024
H = 16
HD = 64
NCORES = 8
SUP = 512           # q super-block width
NSUP = T // SUP
LTN = T // 128      # number of 128-row l-tiles
VSLOT = 130         # v slot: [v_h0(0:64)|1(64)|v_h1(65:129)|1(129)]

F32 = mybir.dt.float32
BF16 = mybir.dt.bfloat16
AF = mybir.ActivationFunctionType
ALU = mybir.AluOpType


def _split_multi_waits(nc, max_waits=1):
    """The walrus build here rejects >1 semaphore wait on one CTRL
    instruction; push excess waits onto preceding same-engine NoOps."""
    n_new = 0
    for f in nc.m.functions:
        for bb in f.blocks:
            out = []
            changed = False
            for ins in bb.instructions:
                si = ins.sync_info
                waits = list(si.on_wait) if si is not None else []
                if len(waits) > max_waits:
                    changed = True
                    excess, keep = waits[:-max_waits], waits[-max_waits:]
                    for ci in range(0, len(excess), max_waits):
                        n_new += 1
                        out.append(mybir.InstNoOp(
                            name=f"{ins.name}-ws{n_new}",
                            engine=ins.engine, ins=[], outs=[],
                            sync_info=mybir.SyncInfo(
                                on_wait=excess[ci:ci + max_waits], on_update=[]),
                        ))
                    ins.sync_info = mybir.SyncInfo(
                        on_wait=keep, on_update=list(si.on_update))
                out.append(ins)
            if changed:
                bb.instructions = out
    return n_new


def build_nc(split_waits=True):
    nc = bass.Bass("TRN2")
    xT = nc.dram_tensor("xT", [C, T], BF16, kind="ExternalInput")
    wq = nc.dram_tensor("wq", [C, 128], BF16, kind="ExternalInput")
    wk = nc.dram_tensor("wk", [C, 128], BF16, kind="ExternalInput")
    wv = nc.dram_tensor("wv", [C, 128], BF16, kind="ExternalInput")
    bq = nc.dram_tensor("bq", [128, 1], F32, kind="ExternalInput")
    bk = nc.dram_tensor("bk", [128, 1], F32, kind="ExternalInput")
    bv = nc.dram_tensor("bv", [128, 1], F32, kind="ExternalInput")
    wp = nc.dram_tensor("wp", [128, C], BF16, kind="ExternalInput")
    out_d = nc.dram_tensor("out", [T, C], BF16, kind="ExternalOutput")

    with tile.TileContext(nc) as tc:
        with ExitStack() as ctx:
            P = lambda **kw: ctx.enter_context(tc.tile_pool(**kw))
            const_p = P(name="const", bufs=1)
            qk_p = P(name="qk", bufs=1)
            v_p = P(name="v", bufs=1)
            x_p = P(name="x", bufs=5)
            vt_p = P(name="vt", bufs=2)
            pt_p = P(name="pt", bufs=6)
            ot_sb_p = P(name="ot_sb", bufs=4)
            ep_p = P(name="ep", bufs=4)
            rl_p = P(name="rl", bufs=2)

            # ---- constants ----
            warm_sb = const_p.tile([128, 64], BF16)
            nc.gpsimd.memset(warm_sb[:], 0.0)
            wq_sb = const_p.tile([128, 8, 128], BF16)
            wk_sb = const_p.tile([128, 8, 128], BF16)
            wv_sb = const_p.tile([128, 8, 128], BF16)
            bq_sb = const_p.tile([128, 1], F32)
            bk_sb = const_p.tile([128, 1], F32)
            bv_sb = const_p.tile([128, 1], F32)
            x_tiles = {}

            def fetch_x(s):
                x_sb = x_p.tile([128, 8, SUP], BF16)
                nc.sync.dma_start(
                    x_sb[:],
                    xT[:, s * SUP:(s + 1) * SUP].rearrange(
                        "(ck p) t -> p ck t", p=128))
                x_tiles[s] = x_sb

            # v-phase runs first, so wv + x(0) must land first; weights go
            # on the ACT queue so their setups overlap the x fetches on the
            # sync queue. x(0) fetched in halves so QKV can start early.
            nc.scalar.dma_start(
                wv_sb[:], wv[:].rearrange("(ck p) m -> p ck m", p=128))
            nc.scalar.dma_start(bv_sb[:], bv[:])
            x0_sb = x_p.tile([128, 8, SUP], BF16)
            for ck in range(8):
                nc.sync.dma_start(
                    x0_sb[:, ck:ck + 1, :],
                    xT[ck * 128:(ck + 1) * 128, 0:SUP].rearrange(
                        "(ck p) t -> p ck t", p=128))
            x_tiles[0] = x0_sb
            nc.scalar.dma_start(
                wq_sb[:], wq[:].rearrange("(ck p) m -> p ck m", p=128))
            nc.scalar.dma_start(bq_sb[:], bq[:])
            nc.scalar.dma_start(
                wk_sb[:], wk[:].rearrange("(ck p) m -> p ck m", p=128))
            nc.scalar.dma_start(bk_sb[:], bk[:])
            fetch_x(1)
            wp_sb = const_p.tile([128, C], BF16)
            nc.scalar.dma_start(wp_sb[:], wp[:])
            ident = const_p.tile([128, 128], BF16)
            make_identity(nc, ident[:])
            ones_bf = const_p.tile([1, 64], BF16)
            nc.gpsimd.memset(ones_bf[:], 1.0)
            v_sb = v_p.tile([128, LTN * VSLOT], BF16)
            nc.gpsimd.memset(v_sb[:], 1.0)  # ones cols survive the transposes

            qT = qk_p.tile([128, T], BF16)
            # per-head kT, zero-padded in the other head's 64 rows, so the
            # S matmul is a full K=128 matmul: transitions between K=64
            # and K=128 weight loads cost ~105-160ns each on the PE (FWL
            # reconfig); with K=128 everywhere every back-to-back matmul
            # runs at the warm ~216ns/512col rate.
            kT0 = qk_p.tile([128, T], BF16)
            kT1 = qk_p.tile([128, T], BF16)
            nc.gpsimd.memset(kT0[64:128, :], 0.0)
            nc.gpsimd.memset(kT1[0:64, :], 0.0)

            st_ps = P(name="st_ps", bufs=2, space="PSUM")
            ot_ps_p = P(name="ot_ps", bufs=2, space="PSUM")
            qkv_ps = P(name="qkv_ps", bufs=1, space="PSUM")
            pj_ps = P(name="pj_ps", bufs=1, space="PSUM")

            qkv_state = {}   # per-super QKV phase state

            def v_thunks(s):
                st = qkv_state.setdefault(s, {})

                def mk(ck):
                    def f():
                        if ck == 0:
                            st["v"] = qkv_ps.tile([128, SUP], F32, tag="qkv",
                                                  name=f"psv{s}")
                        nc.tensor.matmul(
                            st["v"][:], lhsT=wv_sb[:, ck, :],
                            rhs=x_tiles[s][:, ck, :],
                            start=(ck == 0), stop=(ck == 7))
                        if ck == 7:
                            st["vt"] = vt_p.tile([128, SUP], BF16, tag="vt",
                                                 name=f"vt{s}")
                            nc.vector.tensor_scalar_add(
                                out=st["vt"][:], in0=st["v"][:],
                                scalar1=bv_sb[:])
                    return f
                return [mk(c) for c in range(8)]

            def q_thunks(s, dense0=False):
                st = qkv_state.setdefault(s, {})

                def mk(ck):
                    def f():
                        if ck == 0:
                            pool, tg = ((st_ps, "st") if dense0
                                        else (qkv_ps, "qkv"))
                            st["q"] = pool.tile([128, SUP], F32, tag=tg,
                                                name=f"psq{s}")
                        nc.tensor.matmul(
                            st["q"][:], lhsT=wq_sb[:, ck, :],
                            rhs=x_tiles[s][:, ck, :],
                            start=(ck == 0), stop=(ck == 7))
                        if ck == 7:
                            # (q + bias) * 1/sqrt(hd) folded here
                            nc.vector.tensor_scalar(
                                out=qT[:, s * SUP:(s + 1) * SUP],
                                in0=st["q"][:], scalar1=bq_sb[:],
                                scalar2=1.0 / np.sqrt(HD),
                                op0=ALU.add, op1=ALU.mult)
                    return f
                return [mk(c) for c in range(8)]

            def k_thunks(s):
                st = qkv_state.setdefault(s, {})

                def mk(ck):
                    def f():
                        if ck == 0:
                            st["k"] = pj_ps.tile([128, SUP], F32, tag="pj",
                                                 name=f"psk{s}")
                        nc.tensor.matmul(
                            st["k"][:], lhsT=wk_sb[:, ck, :],
                            rhs=x_tiles[s][:, ck, :],
                            start=(ck == 0), stop=(ck == 7))
                        if ck == 7:
                            nc.vector.tensor_scalar_add(
                                out=kT0[0:64, s * SUP:(s + 1) * SUP],
                                in0=st["k"][0:64, :], scalar1=bk_sb[0:64])
                            nc.vector.tensor_scalar_add(
                                out=kT1[64:128, s * SUP:(s + 1) * SUP],
                                in0=st["k"][64:128, :],
                                scalar1=bk_sb[64:128])
                            x_tiles.pop(s)
                    return f
                return [mk(c) for c in range(8)]

            def tp_thunks(s, dense0=False):
                st = qkv_state.setdefault(s, {})

                def mk(lt_loc):
                    def f():
                        if lt_loc == 0:
                            pool, tg = ((st_ps, "st") if dense0
                                        else (qkv_ps, "qkv"))
                            st["tp"] = pool.tile([128, SUP], BF16, tag=tg,
                                                 name=f"tp{s}")
                        lt = s * 4 + lt_loc
                        blk = slice(lt_loc * 128, (lt_loc + 1) * 128)
                        nc.tensor.transpose(
                            st["tp"][:, blk], st["vt"][:, blk], ident[:])
                        nc.vector.tensor_copy(
                            v_sb[:, lt * VSLOT: lt * VSLOT + 64],
                            st["tp"][:, lt_loc * 128: lt_loc * 128 + 64])
                        nc.vector.tensor_copy(
                            v_sb[:, lt * VSLOT + 65: lt * VSLOT + 129],
                            st["tp"][:, lt_loc * 128 + 64:
                                      lt_loc * 128 + 128])
                        if lt_loc == 3:
                            qkv_state.pop(s, None)
                    return f
                return [mk(b) for b in range(4)]

            def proj_thunks(jj, ot_sb, alternate=False):
                """Projection for super jj: 8 single-MM thunks, each with
                its cast + store. One psum bank; spacing covers the WAR.
                alternate=True (epilogue): rotate a 2nd (st) bank and split
                casts DVE/ACT so the tail isn't serialized on one drain."""
                def mk(idx, tb, half):
                    def f():
                        if alternate and idx % 2:
                            pj = st_ps.tile([128, 512], F32, tag="st",
                                            name=f"pj{jj}_{tb}_{half}")
                        else:
                            pj = pj_ps.tile([128, 512], F32, tag="pj",
                                            name=f"pj{jj}_{tb}_{half}")
                        nc.tensor.matmul(
                            pj[:],
                            lhsT=ot_sb[:, tb * 128:(tb + 1) * 128],
                            rhs=wp_sb[:, half * 512:(half + 1) * 512],
                            start=True, stop=True)
                        res = ep_p.tile([128, 512], BF16, tag="res",
                                        name=f"res{jj}_{tb}_{half}")
                        if alternate and idx % 2:
                            nc.scalar.activation(res[:], pj[:], AF.Copy)
                        else:
                            nc.vector.tensor_copy(res[:], pj[:])
                        nc.sync.dma_start(
                            out_d[jj * SUP + tb * 128:
                                  jj * SUP + (tb + 1) * 128,
                                  half * 512:(half + 1) * 512],
                            res[:])
                    return f
                return [mk(i, i // 2, i % 2) for i in range(8)]

            # ---- HAM warm-up: dependency-free tiny matmuls fill the
            # initial x-chunk DMA gaps so the PE clock gate reaches K=8/8
            # (2.4 GHz) before the main QKV stream instead of ~3.4us in;
            # interleaved between the v thunks so they plug per-chunk
            # DMA waits rather than delaying ready work ----
            warm_ps = pj_ps.tile([64, 64], F32, tag="pj", name="warm_ps")

            def warm(nmm):
                for wi in range(nmm):
                    nc.tensor.matmul(warm_ps[:], lhsT=warm_sb[:, 0:64],
                                     rhs=warm_sb[:, 0:64],
                                     start=True, stop=True)

            # ---- prologue: QKV(0) dense (q/tp borrow st banks so the
            # qkv bank has no WAR stalls), then prefetch ----
            warm(10)
            vth = v_thunks(0)
            for i, th in enumerate(vth):
                th()
                if i < 5:
                    warm(6)
            for th in (q_thunks(0, dense0=True) + k_thunks(0)
                       + tp_thunks(0, dense0=True)):
                th()
            fetch_x(2)
            fetch_x(3)

            ot_sb_tiles = {}     # j -> normalized [128, SUP] bf16
            pending_norm = {}    # j -> thunk emitting the 2 norm mults
            pending_bc = {}      # j -> thunk emitting h1's bc matmul+copy

            for j in range(NSUP):
                nlt = 4 * j + 4
                # head-blocked pair order: 2 l-tile pairs per head block.
                # Pair granularity on the exp: ACT has ~250-350ns of fixed
                # per-op overhead, so one [128,1024] exp beats two
                # [128,512] ones by ~300ns.
                pairs = []
                # head-blocks of 8 l-tiles (plus a remainder block of 4):
                # bigger blocks push each super's first h1 PV later (more
                # cover for the previous super's norm chain) and start h0's
                # den earlier.
                blks = [4] * (nlt // 4)
                t0b = 0
                for hb_ in blks:
                    for h in range(2):
                        for q2 in range(hb_ // 2):
                            pairs.append((t0b + 2 * q2, h))
                    t0b += hb_
                n = len(pairs)
                hb = blks[-1]
                last = j == NSUP - 1

                ot_ps = [ot_ps_p.tile([128, SUP], F32, tag="ot",
                                      name=f"ot{j}_{hh}") for hh in range(2)]
                rcs = [rl_p.tile([1, SUP], BF16, tag=f"rc{hh}",
                                 name=f"rc{j}_{hh}") for hh in range(2)]
                lns = [rl_p.tile([1, SUP], F32, tag=f"ln{hh}",
                                 name=f"ln{j}_{hh}") for hh in range(2)]
                bc_sb = rl_p.tile([128, SUP], BF16, tag="bc_sb",
                                  name=f"bc_sb{j}")
                ot_sb = ot_sb_p.tile([128, SUP], BF16, tag="ot_sb",
                                     name=f"ot_sb{j}")
                ot_sb_tiles[j] = ot_sb

                # ---- filler schedule for this super ----
                per = [[] for _ in range(n)]
                fillers = []

                def sched(thunks, lo, hi=None, dense=None):
                    """Spread thunks over iters lo..hi (dense = per-iter)."""
                    hi = n if hi is None else hi
                    base = len(fillers)
                    fillers.extend(thunks)
                    m = len(thunks)
                    if dense:
                        for i in range(m):
                            per[min(lo + i // dense, n - 1)].append(base + i)
                    else:
                        for i in range(m):
                            per[min(lo + i * (hi - lo) // m, n - 1)].append(
                                base + i)

                if j >= 1:
                    sched(v_thunks(j), 0, dense=8)     # iter 0, all slot-a
                    sched(k_thunks(j), 1, dense=8)     # iter 1 (pj bank)
                    sched(tp_thunks(j), 2, dense=4)    # iter 2
                if j + 1 < NSUP:
                    sched(q_thunks(j + 1), 3 if j >= 1 else 0)
                if j >= 1:
                    sched(proj_thunks(j - 1, ot_sb_tiles.pop(j - 1)), 4)

                def emit_S(p, idx):
                    t0, h = pairs[p]
                    t = t0 + idx
                    # second tile of a diagonal pair is NOT trimmed: the
                    # extra S cols land in the psum gap so one exp covers
                    # the whole pair (ACT per-op overhead ~0.7us dwarfs the
                    # ~0.1-0.2us of extra PE cols); the gap region is never
                    # read by PV.
                    n0 = max(0, 128 * (t - 4 * j)) if idx == 0 else 0
                    if idx == 0:
                        pair_state[p] = [
                            st_ps.tile([128, 2 * SUP], F32, tag="st",
                                       name=f"st{j}_{t0}_{h}"), None]
                    stt = pair_state[p][0]
                    kTh = kT0 if h == 0 else kT1
                    nc.tensor.matmul(
                        stt[:, idx * SUP + n0:(idx + 1) * SUP],
                        lhsT=kTh[:, t * 128:(t + 1) * 128],
                        rhs=qT[:, j * SUP + n0:(j + 1) * SUP],
                        start=True, stop=True)

                def emit_exp(p):
                    t0, h = pairs[p]
                    stt = pair_state[p][0]
                    pt = pt_p.tile([128, 2 * SUP], BF16, tag="pt",
                                   name=f"pt{j}_{t0}_{h}")
                    pair_state[p][1] = pt
                    e0 = max(0, 128 * (t0 - 4 * j))
                    nc.scalar.activation(
                        pt[:, e0:2 * SUP], stt[:, e0:2 * SUP], AF.Exp)
                    for idx in range(2):
                        t = t0 + idx
                        if t >= 4 * j:
                            # zero strictly-upper triangle of the diagonal
                            # 128-col block: keep col >= part
                            n0 = 128 * (t - 4 * j)
                            nc.gpsimd.affine_select(
                                out=pt[:, idx * SUP + n0:
                                       idx * SUP + n0 + 128],
                                in_=pt[:, idx * SUP + n0:
                                       idx * SUP + n0 + 128],
                                compare_op=ALU.is_ge, fill=0.0, base=0,
                                channel_multiplier=-1, pattern=[[1, 128]])

                def emit_PV(p, idx):
                    t0, h = pairs[p]
                    t = t0 + idx
                    n0 = max(0, 128 * (t - 4 * j))
                    pt = pair_state[p][1]
                    nc.tensor.matmul(
                        ot_ps[h][0:65, n0:SUP],
                        lhsT=v_sb[:, t * VSLOT + h * 65:
                                  t * VSLOT + (h + 1) * 65],
                        rhs=pt[:, idx * SUP + n0:(idx + 1) * SUP],
                        start=(t == 0), stop=(t == 4 * j + 3))

                def den_chain(h):
                    # 1/den as exp(-ln den) on ACT (~0.8us/op; Ln/Exp share
                    # one table; DVE reciprocal measures 3.3us and a DVE
                    # divide does not exist on this HW). rc is written bf16
                    # for the PE ones-matmul broadcast.
                    nc.scalar.activation(lns[h][:], ot_ps[h][64:65, :],
                                         AF.Ln)
                    nc.scalar.activation(rcs[h][:], lns[h][:], AF.Exp,
                                         scale=-1.0)

                def bc_mm(h, bct, rcs_, bc_sb_):
                    # broadcast 1/den across partitions with a K=1 bf16
                    # ones-matmul (~0.2us on PE) + DVE copy to SBUF -
                    # ~2.5us shorter chain than a DRAM-roundtrip DMA.
                    nc.tensor.matmul(
                        bct[h * 64:(h + 1) * 64, :], lhsT=ones_bf[:],
                        rhs=rcs_[h][:], start=True, stop=True,
                        tile_position=(0, h * 64))
                    nc.vector.tensor_copy(
                        bc_sb_[h * 64:(h + 1) * 64, :],
                        bct[h * 64:(h + 1) * 64, :])

                def make_norm(jj, ot_ps_, bc_, ot_sb_):
                    def f():
                        for h in range(2):
                            nc.vector.tensor_tensor(
                                out=ot_sb_[h * 64:(h + 1) * 64, :],
                                in0=ot_ps_[h][0:64, :],
                                in1=bc_[h * 64:(h + 1) * 64, :],
                                op=ALU.mult)
                    return f

                pair_state = {}
                emit_S(0, 0)
                emit_S(0, 1)
                emit_exp(0)
                for p in range(n):
                    if p == 0 and j + 4 < NSUP:
                        fetch_x(j + 4)
                    if p == 1 and j >= 1:
                        pending_norm.pop(j - 1)()
                    fa = per[p]
                    # iter 0: all fillers before the PVs - they are the PE
                    # cover for the previous super's den/norm chain, which
                    # the first PVs (ot-bank WAR) wait on
                    half = len(fa) if p == 0 else (len(fa) + 1) // 2
                    # full S pair + exp first so ACT gets a ~full-iter
                    # head start on exp(p+1) before PV(p+1,*) need it
                    if p + 1 < n:
                        emit_S(p + 1, 0)
                        emit_S(p + 1, 1)
                        emit_exp(p + 1)
                    for fi in fa[:half]:
                        fillers[fi]()
                    if p == 0 and j >= 1:
                        pending_bc.pop(j - 1)()
                    emit_PV(p, 0)
                    for fi in fa[half:]:
                        fillers[fi]()
                    emit_PV(p, 1)
                    pair_state.pop(p)
                    if p == n - 1 - hb // 2:
                        den_chain(0)
                    if p == n - 1:
                        den_chain(1)

                        def mk_bc(f=bc_mm, jj=j, r=rcs, b=bc_sb):
                            def g():
                                bct = pj_ps.tile([128, SUP], F32, tag="pj",
                                                 name=f"bct{jj}")
                                f(0, bct, r, b)
                                f(1, bct, r, b)
                            return g
                        pending_bc[j] = mk_bc()
                        pending_norm[j] = make_norm(j, ot_ps, bc_sb, ot_sb)

            # ---- epilogue: last super's norm + projection, interleaved
            # by 128-col blocks so each projection matmul starts as soon as
            # its slice of the normalize is done ----
            jl = NSUP - 1
            del pending_norm[jl]
            pending_bc.pop(jl)()
            pths = proj_thunks(jl, ot_sb_tiles.pop(jl), alternate=True)
            for tb in range(4):
                cs = slice(tb * 128, (tb + 1) * 128)
                for h in range(2):
                    nc.vector.tensor_tensor(
                        out=ot_sb[h * 64:(h + 1) * 64, cs],
                        in0=ot_ps[h][0:64, cs],
                        in1=bc_sb[h * 64:(h + 1) * 64, cs],
                        op=ALU.mult)
                pths[2 * tb]()
                pths[2 * tb + 1]()

    if split_waits:
        _split_multi_waits(nc, 1)
    return nc


_NC_CACHE = {}


def _get_nc():
    if "nc" not in _NC_CACHE:
        _NC_CACHE["nc"] = build_nc()
    return _NC_CACHE["nc"]


def make_in_maps(x, W_attn, b_attn, W_proj, b_proj):
    import ml_dtypes
    bf = ml_dtypes.bfloat16
    x = np.ascontiguousarray(np.asarray(x, dtype=np.float32)).reshape(T, C)
    W_attn = np.asarray(W_attn, dtype=np.float32)
    b_attn = np.asarray(b_attn, dtype=np.float32)
    W_proj = np.asarray(W_proj, dtype=np.float32)
    xT = np.ascontiguousarray(x.T).astype(bf)
    in_maps = []
    for c in range(NCORES):
        sl = slice(128 * c, 128 * (c + 1))
        m = {
            "xT": xT,
            "wq": np.ascontiguousarray(W_attn[:, sl]).astype(bf),
            "wk": np.ascontiguousarray(W_attn[:, C:][:, sl]).astype(bf),
            "wv": np.ascontiguousarray(W_attn[:, 2 * C:][:, sl]).astype(bf),
            "bq": np.ascontiguousarray(b_attn[sl]).reshape(128, 1),
            "bk": np.ascontiguousarray(b_attn[C:][sl]).reshape(128, 1),
            "bv": np.ascontiguousarray(b_attn[2 * C:][sl]).reshape(128, 1),
            "wp": np.ascontiguousarray(W_proj[sl, :]).astype(bf),
        }
        in_maps.append(m)
    return in_maps


def kernel(x, W_attn, b_attn, W_proj, b_proj):
    from concourse.bass_utils import run_bass_kernel_spmd
    nc = _get_nc()
    in_maps = make_in_maps(x, W_attn, b_attn, W_proj, b_proj)
    res = run_bass_kernel_spmd(nc, in_maps, core_ids=list(range(NCORES)))
    acc = np.zeros((T, C), dtype=np.float32)
    for c in range(NCORES):
        acc += np.asarray(res.results[c]["out"], dtype=np.float32)
    acc += np.asarray(b_proj, dtype=np.float32)  # bias folded into unshard
    return acc.reshape(1, T, C)
